# revision 4
# baseline (speedup 1.0000x reference)
"""NeighborhoodShift2d: stack 49 spatially shifted (zero-padded) copies.

Input  x:  [1, 8, 32, 128, 128]  (B, heads, dim, H, W) fp32
Output y:  [1, 8, 49, 32, 128, 128]  y[:, :, k] = shift(x, OFFSETS[k]) with
zero padding, k enumerating the 7x7 NATTEN stencil (dy major, dx minor).

Sharding: pure data-parallel, one head per NeuronCore (8 heads, 8 cores).

Per-core program (pure DMA, memory-bound):
  - Stage the head in SBUF: partition p = image row y (128 rows), free dim
    [c: 32, w': 134] with 3 zero-pad columns each side (handles dx shifts).
  - For each of 49 offsets, one ~2 MiB HWDGE DMA: strided SBUF read
    (partitions y+dy, cols 3+dx..) -> strided DRAM write in (y, c, x) order.
  - Edge rows that fall outside the image (|dy| rows) are zero-filled from a
    zero SBUF tile with one DMA per dy value (the 7 dx offsets for a given dy
    are k-consecutive in the output, and (k, c) dims merge).
"""

import numpy as np

import concourse.bass as bass
import concourse.mybir as mybir
from concourse.bass_utils import run_bass_kernel_spmd

B, HEADS, C, H, W = 1, 8, 32, 128, 128
WIN = 7
PAD = 3
K = WIN * WIN
WP = W + 2 * PAD  # 134

_nc_cache = None


def _offsets():
    """(k, dy, dx) for the 7x7 stencil, k in output order."""
    out = []
    for dy in range(-PAD, PAD + 1):
        for dx in range(-PAD, PAD + 1):
            out.append(((dy + PAD) * WIN + (dx + PAD), dy, dx))
    return out


def _build_nc():
    nc = bass.Bass()
    x = nc.dram_tensor("x", [C, H, W], mybir.dt.float32, kind="ExternalInput")
    y = nc.dram_tensor("y", [K, C, H, W], mybir.dt.float32, kind="ExternalOutput")

    # Split DMAs between the two HWDGE rings (SP="sync", ACT="scalar").
    interior = _offsets()
    edges = [(k, dy, dx) for (k, dy, dx) in interior if dy != 0]

    with (
        nc.sbuf_tensor("P", [H, C, WP], mybir.dt.float32) as P,
        nc.sbuf_tensor("Z", [C, PAD * W], mybir.dt.float32) as Z,
        nc.semaphore("s_dve") as s_dve,
        nc.semaphore("s_load") as s_load,
        nc.semaphore("s_sp") as s_sp,
        nc.semaphore("s_act") as s_act,
        nc.Block() as block,
    ):
        n_sp = [0]
        n_act = [0]

        def store(eng, dst, src):
            if eng is nc.sync:
                n_sp[0] += 1
                eng.dma_start(out=dst, in_=src).then_inc(s_sp, 16)
            else:
                n_act[0] += 1
                eng.dma_start(out=dst, in_=src).then_inc(s_act, 16)

        @block.vector
        def _(vector):
            # Zero the dx pad columns of P and the zero-fill tile Z.
            vector.memset(P[:, :, 0:PAD], 0.0).then_inc(s_dve, 1)
            vector.memset(P[:, :, WP - PAD : WP], 0.0).then_inc(s_dve, 1)
            vector.memset(Z[:, :], 0.0).then_inc(s_dve, 1)

        @block.sync
        def _(sync):
            # Load head into padded SBUF image, (y, c, x) element order.
            sync.dma_start(
                out=P[:, :, PAD : PAD + W],
                in_=x[:, :, :].transpose([1, 0, 2]),
            ).then_inc(s_load, 16)

            sync.wait_ge(s_dve, 3)
            # Edge-row zero fills (even k) — only need Z.
            for (k, dy, dx) in edges:
                if k % 2 != 0:
                    continue
                g = abs(dy)
                r0 = 0 if dy < 0 else H - g
                store(nc.sync, y[k, :, r0 : r0 + g, :], Z[:, 0 : g * W])

            sync.wait_ge(s_load, 16)
            # Interior shifted copies (even k), ~2 MiB each.
            for (k, dy, dx) in interior:
                if k % 2 != 0:
                    continue
                n = H - abs(dy)
                ys_src = max(0, dy)
                ys_dst = max(0, -dy)
                store(
                    nc.sync,
                    y[k, :, ys_dst : ys_dst + n, :].transpose([1, 0, 2]),
                    P[ys_src : ys_src + n, :, PAD + dx : PAD + dx + W],
                )
            sync.wait_ge(s_sp, 16 * n_sp[0])

        @block.scalar
        def _(scalar):
            scalar.wait_ge(s_dve, 3)
            for (k, dy, dx) in edges:
                if k % 2 != 1:
                    continue
                g = abs(dy)
                r0 = 0 if dy < 0 else H - g
                store(nc.scalar, y[k, :, r0 : r0 + g, :], Z[:, 0 : g * W])

            scalar.wait_ge(s_load, 16)
            for (k, dy, dx) in interior:
                if k % 2 != 1:
                    continue
                n = H - abs(dy)
                ys_src = max(0, dy)
                ys_dst = max(0, -dy)
                store(
                    nc.scalar,
                    y[k, :, ys_dst : ys_dst + n, :].transpose([1, 0, 2]),
                    P[ys_src : ys_src + n, :, PAD + dx : PAD + dx + W],
                )
            scalar.wait_ge(s_act, 16 * n_act[0])

    return nc


def _get_nc():
    global _nc_cache
    if _nc_cache is None:
        _nc_cache = _build_nc()
    return _nc_cache


def kernel(x: np.ndarray) -> np.ndarray:
    assert x.shape == (B, HEADS, C, H, W), x.shape
    nc = _get_nc()
    in_maps = [
        {"x": np.ascontiguousarray(x[0, h], dtype=np.float32)} for h in range(HEADS)
    ]
    res = run_bass_kernel_spmd(nc, in_maps, core_ids=list(range(HEADS)))
    out = np.stack([res.results[h]["y"] for h in range(HEADS)], axis=0)
    return out[None]  # [1, 8, 49, 32, 128, 128]


# revision 5
# speedup vs baseline: 3.8053x; 3.8053x over previous
"""NeighborhoodShift2d: stack 49 spatially shifted (zero-padded) copies.

Input  x:  [1, 8, 32, 128, 128]  (B, heads, dim, H, W) fp32
Output y:  [1, 8, 49, 32, 128, 128]  y[:, :, k] = shift(x, OFFSETS[k]) with
zero padding, k enumerating the 7x7 NATTEN stencil (dy major, dx minor).

Sharding: pure data-parallel, one head per NeuronCore (8 heads, 8 cores).

Per-core program (pure DMA, memory-bound). Trick: SDMA throughput is
per-descriptor-bound (~40 ns/descriptor), so descriptors must be large.
We bake the dx shift into SBUF: 7 pre-shifted flat copies of the head,
each with the dx zero columns baked in:
  BufA[32*i + c] = flat 128x128 image of channel c shifted by dx = i-3
  BufB[32*b + c] = shifted by dx = b+1
Then for each dy, the 7 dx output blocks (consecutive k) are written by
just TWO DMAs (BufA -> 4 k-blocks, BufB -> 3 k-blocks), each descriptor
a fully contiguous (H-|dy|)*128 float run per partition (up to 64 KiB).
Edge rows (|dy| rows outside the image) are zero-filled from a zero tile
with one batched DMA per dy.
"""

import numpy as np

import concourse.bass as bass
import concourse.mybir as mybir
from concourse.bass_utils import run_bass_kernel_spmd

B, HEADS, C, H, W = 1, 8, 32, 128, 128
WIN = 7
PAD = 3
K = WIN * WIN

_nc_cache = None


def _build_nc():
    f32 = mybir.dt.float32
    nc = bass.Bass()
    x = nc.dram_tensor("x", [C, H, W], f32, kind="ExternalInput")
    y = nc.dram_tensor("y", [K, C, H, W], f32, kind="ExternalOutput")

    with (
        nc.sbuf_tensor("BufA", [4 * C, H, W], f32) as BufA,  # dx = -3..0
        nc.sbuf_tensor("BufB", [3 * C, H, W], f32) as BufB,  # dx = +1..+3
        nc.sbuf_tensor("Z", [112, 768], f32) as Z,
        nc.semaphore("s_dve") as s_dve,
        nc.semaphore("s_load") as s_load,
        nc.semaphore("s_sp") as s_sp,
        nc.semaphore("s_act") as s_act,
        nc.Block() as block,
    ):
        def band(dx):
            """(buffer, first partition) holding the dx-shifted copy."""
            return (BufA, 32 * (dx + 3)) if dx <= 0 else (BufB, 32 * (dx - 1))

        @block.vector
        def _(vector):
            # Bake the dx zero columns; zero the edge-row source tile.
            for dx in range(-PAD, PAD + 1):
                if dx == 0:
                    continue
                buf, p0 = band(dx)
                if dx < 0:
                    ap = buf[p0 : p0 + C, :, 0:-dx]
                else:
                    ap = buf[p0 : p0 + C, :, W - dx : W]
                vector.memset(ap, 0.0).then_inc(s_dve, 1)
            vector.memset(Z[:, :], 0.0).then_inc(s_dve, 1)

        def loads(eng, dxs):
            for dx in dxs:
                buf, p0 = band(dx)
                m = W - abs(dx)
                xd = max(0, -dx)  # dst col range [xd, xd+m)
                xs = max(0, dx)   # src col range [xs, xs+m)
                eng.dma_start(
                    out=buf[p0 : p0 + C, :, xd : xd + m],
                    in_=x[:, :, xs : xs + m],
                ).then_inc(s_load, 16)

        def edge(eng, dy, sem):
            g = abs(dy)
            k0 = (dy + PAD) * WIN
            r0 = 0 if dy < 0 else H - g
            eng.dma_start(
                out=y[k0 : k0 + WIN, :, r0 : r0 + g, :],
                in_=bass.AP(Z, 0, [[768, 112], [384, 2], [1, 128 * g]]),
            ).then_inc(sem, 16)

        def store(eng, dy, which, sem):
            n = H - abs(dy)
            ys = max(0, dy)    # first source row
            yd = max(0, -dy)   # first dst row
            k0 = (dy + PAD) * WIN
            if which == "A":
                src = BufA[:, ys : ys + n, :]
                dst = y[k0 : k0 + 4, :, yd : yd + n, :]
            else:
                src = BufB[:, ys : ys + n, :]
                dst = y[k0 + 4 : k0 + WIN, :, yd : yd + n, :]
            eng.dma_start(out=dst, in_=src).then_inc(sem, 16)

        # Alternate A/B between the queues per dy to balance bytes.
        dys = list(range(-PAD, PAD + 1))
        sync_stores = [(dy, "A" if i % 2 == 0 else "B") for i, dy in enumerate(dys)]
        act_stores = [(dy, "B" if i % 2 == 0 else "A") for i, dy in enumerate(dys)]

        @block.sync
        def _(sync):
            loads(nc.sync, [-3, -2, -1, 0])
            sync.wait_ge(s_dve, 7)
            for dy in (-3, -2, -1):
                edge(nc.sync, dy, s_sp)
            sync.wait_ge(s_load, 16 * 7)
            for dy, which in sync_stores:
                store(nc.sync, dy, which, s_sp)
            sync.wait_ge(s_sp, 16 * (3 + len(sync_stores)))

        @block.scalar
        def _(scalar):
            loads(nc.scalar, [1, 2, 3])
            scalar.wait_ge(s_dve, 7)
            for dy in (1, 2, 3):
                edge(nc.scalar, dy, s_act)
            scalar.wait_ge(s_load, 16 * 7)
            for dy, which in act_stores:
                store(nc.scalar, dy, which, s_act)
            scalar.wait_ge(s_act, 16 * (3 + len(act_stores)))

    return nc


def _get_nc():
    global _nc_cache
    if _nc_cache is None:
        _nc_cache = _build_nc()
    return _nc_cache


def kernel(x: np.ndarray) -> np.ndarray:
    assert x.shape == (B, HEADS, C, H, W), x.shape
    nc = _get_nc()
    in_maps = [
        {"x": np.ascontiguousarray(x[0, h], dtype=np.float32)} for h in range(HEADS)
    ]
    res = run_bass_kernel_spmd(nc, in_maps, core_ids=list(range(HEADS)))
    out = np.stack([res.results[h]["y"] for h in range(HEADS)], axis=0)
    return out[None]  # [1, 8, 49, 32, 128, 128]


# revision 6
# speedup vs baseline: 4.2490x; 1.1166x over previous
"""NeighborhoodShift2d: stack 49 spatially shifted (zero-padded) copies.

Input  x:  [1, 8, 32, 128, 128]  (B, heads, dim, H, W) fp32
Output y:  [1, 8, 49, 32, 128, 128]  y[:, :, k] = shift(x, OFFSETS[k]) with
zero padding, k enumerating the 7x7 NATTEN stencil (dy major, dx minor).

Sharding: pure data-parallel, one head per NeuronCore (8 heads, 8 cores).

Per-core program (pure DMA, memory-bound). Design notes:
- SDMA throughput is per-descriptor-bound, so descriptors must be big.
  The dx shift is baked into SBUF as 7 pre-shifted flat copies (zero
  columns included); a store descriptor is then a fully contiguous
  (H-|dy|)*W float run per channel (up to 64 KiB).
- SBUF AXI ports: partitions [0,64) sit on the 8 even ports, [64,128) on
  the 8 odd ports (~218 GB/s per parity). Bands are placed so each
  parity carries exactly half the store traffic, and the two HWDGE
  queues (sync=SP, scalar=ACT) are pinned to opposite parities. The
  dx=0 image is kept twice (T1[96:128] odd, T2[0:32] even) and its
  stores alternate parity by dy.
- T1 bands (by partition / 32): [-3, -2, -1, 0];  T2: [0 dup, +1, +2, +3]
- Loads: two flat 2 MiB DMAs (dx=0 into T1[96:128] and T2[0:32]); the
  six shifted copies are SBUF->SBUF star copies off the two dup bands
  (a flat copy shifted by dx; wrap garbage lands in the dx zero columns,
  which DVE memsets afterwards).
- Edge rows (|dy| rows outside the image) are zero-filled from a zero
  tile with one batched DMA per dy (7 k-blocks at once).
"""

import numpy as np

import concourse.bass as bass
import concourse.mybir as mybir
from concourse.bass_utils import run_bass_kernel_spmd

B, HEADS, C, H, W = 1, 8, 32, 128, 128
WIN = 7
PAD = 3
K = WIN * WIN
FP = H * W  # flat image floats per partition (16384)

_nc_cache = None


def _build_nc():
    f32 = mybir.dt.float32
    nc = bass.Bass()
    x = nc.dram_tensor("x", [C, H, W], f32, kind="ExternalInput")
    y = nc.dram_tensor("y", [K, C, H, W], f32, kind="ExternalOutput")

    with (
        nc.sbuf_tensor("T1", [4 * C, H, W], f32) as T1,
        nc.sbuf_tensor("T2", [4 * C, H, W], f32) as T2,
        nc.sbuf_tensor("Z", [112, 768], f32) as Z,
        nc.semaphore("s_ld") as s_ld,
        nc.semaphore("s_cpA") as s_cpA,
        nc.semaphore("s_cpB") as s_cpB,
        nc.semaphore("s_dve") as s_dve,
        nc.semaphore("s_sp") as s_sp,
        nc.semaphore("s_act") as s_act,
        nc.Block() as block,
    ):
        def band(dx, dy=0):
            """(tensor, first partition) of the dx-shifted flat copy."""
            if dx < 0:
                return T1, 32 * (dx + 3)
            if dx > 0:
                return T2, 32 * dx
            return (T2, 0) if dy % 2 == 0 else (T1, 96)

        @block.vector
        def _(vector):
            vector.memset(Z[:, :], 0.0).then_inc(s_dve, 1)
            vector.wait_ge(s_cpA, 48)
            vector.wait_ge(s_cpB, 48)
            # Bake the dx zero columns (also covers star-copy wrap garbage).
            for dx in range(-PAD, PAD + 1):
                if dx == 0:
                    continue
                buf, p0 = band(dx)
                if dx < 0:
                    ap = buf[p0 : p0 + C, :, 0:-dx]
                else:
                    ap = buf[p0 : p0 + C, :, W - dx : W]
                vector.memset(ap, 0.0).then_inc(s_dve, 1)

        def copy_shift(eng, dx, sem):
            """band(dx) = dup image flat-shifted by dx (SBUF->SBUF)."""
            buf, p0 = band(dx)
            src_buf, sp0 = (T2, 0) if dx > 0 else (T1, 96)
            if dx > 0:
                dst = bass.AP(buf, p0 * FP, [[FP, C], [1, FP - dx]])
                src = bass.AP(src_buf, sp0 * FP + dx, [[FP, C], [1, FP - dx]])
            else:
                dst = bass.AP(buf, p0 * FP - dx, [[FP, C], [1, FP + dx]])
                src = bass.AP(src_buf, sp0 * FP, [[FP, C], [1, FP + dx]])
            eng.dma_start(out=dst, in_=src).then_inc(sem, 16)

        def edge(eng, dy, sem):
            g = abs(dy)
            k0 = (dy + PAD) * WIN
            r0 = 0 if dy < 0 else H - g
            eng.dma_start(
                out=y[k0 : k0 + WIN, :, r0 : r0 + g, :],
                in_=bass.AP(Z, 0, [[768, 112], [384, 2], [1, 128 * g]]),
            ).then_inc(sem, 16)

        def store(eng, dy, dx, sem):
            n = H - abs(dy)
            ys, yd = max(0, dy), max(0, -dy)
            k = (dy + PAD) * WIN + (dx + PAD)
            buf, p0 = band(dx, dy)
            src = bass.AP(buf, p0 * FP + ys * W, [[FP, C], [1, n * W]])
            dst = y[k, :, yd : yd + n, :]
            eng.dma_start(out=dst, in_=src).then_inc(sem, 16)

        # Parity split: even ports = partitions < 64.
        def is_even(dy, dx):
            _, p0 = band(dx, dy)
            return p0 < 64

        dys = list(range(-PAD, PAD + 1))
        sync_stores = [(dy, dx) for dy in dys for dx in dys if is_even(dy, dx)]
        act_stores = [(dy, dx) for dy in dys for dx in dys if not is_even(dy, dx)]

        @block.sync
        def _(sync):
            # flat dx=0 load into T1[96:128] (odd-port dup)
            sync.dma_start(out=T1[96:128, :, :], in_=x[:, :, :]).then_inc(s_ld, 16)
            sync.wait_ge(s_dve, 1)
            for dy in (-3, -2, -1):
                edge(nc.sync, dy, s_sp)
            sync.wait_ge(s_ld, 32)
            for dx in (1, 2, 3):
                copy_shift(nc.sync, dx, s_cpB)
            sync.wait_ge(s_dve, 7)
            for dy, dx in sync_stores:
                store(nc.sync, dy, dx, s_sp)
            sync.wait_ge(s_sp, 16 * (3 + len(sync_stores)))

        @block.scalar
        def _(scalar):
            # flat dx=0 load into T2[0:32] (even-port dup)
            scalar.dma_start(out=T2[0:32, :, :], in_=x[:, :, :]).then_inc(s_ld, 16)
            scalar.wait_ge(s_dve, 1)
            for dy in (1, 2, 3):
                edge(nc.scalar, dy, s_act)
            scalar.wait_ge(s_ld, 32)
            for dx in (-1, -2, -3):
                copy_shift(nc.scalar, dx, s_cpA)
            scalar.wait_ge(s_dve, 7)
            for dy, dx in act_stores:
                store(nc.scalar, dy, dx, s_act)
            scalar.wait_ge(s_act, 16 * (3 + len(act_stores)))

    return nc


def _get_nc():
    global _nc_cache
    if _nc_cache is None:
        _nc_cache = _build_nc()
    return _nc_cache


def kernel(x: np.ndarray) -> np.ndarray:
    assert x.shape == (B, HEADS, C, H, W), x.shape
    nc = _get_nc()
    in_maps = [
        {"x": np.ascontiguousarray(x[0, h], dtype=np.float32)} for h in range(HEADS)
    ]
    res = run_bass_kernel_spmd(nc, in_maps, core_ids=list(range(HEADS)))
    out = np.stack([res.results[h]["y"] for h in range(HEADS)], axis=0)
    return out[None]  # [1, 8, 49, 32, 128, 128]


# revision 7
# speedup vs baseline: 5.2032x; 1.2246x over previous
"""NeighborhoodShift2d: stack 49 spatially shifted (zero-padded) copies.

Input  x:  [1, 8, 32, 128, 128]  (B, heads, dim, H, W) fp32
Output y:  [1, 8, 49, 32, 128, 128]  y[:, :, k] = shift(x, OFFSETS[k]) with
zero padding, k enumerating the 7x7 NATTEN stencil (dy major, dx minor).

Sharding: pure data-parallel, one head per NeuronCore (8 heads, 8 cores).

Per-core program (pure DMA, memory-bound). Design notes:
- SDMA throughput is per-descriptor-bound, so every transfer uses big
  contiguous descriptors (up to 64 KiB). The dx shift is baked into SBUF
  as 7 flat per-channel image copies, each loaded DIRECTLY from DRAM as a
  flat shifted window (x[c].flat[dx:FP] is contiguous!). The |dx| wrap
  columns (row-boundary wrap garbage / dx zero padding) are then zeroed
  by DVE memsets. A store descriptor is a fully contiguous
  (H-|dy|)*W-float run per channel.
- SBUF AXI port parity: partitions [0,64) sit on the 8 even ports,
  [64,128) on the 8 odd ports (~218 GB/s per parity). Bands are placed
  so each parity carries half the store traffic; the sync (SP) queue
  issues only even-parity-band stores, scalar (ACT) only odd. The dx=0
  image is kept twice (T1[96:128] odd, T2[0:32] even) and its stores
  alternate parity by dy.
- T1 bands (by partition/32): [-3, -2, -1, 0dup]; T2: [0dup, +1, +2, +3]
- Stores are gated per band (semaphore thresholds) so they start as soon
  as that band's load+memset landed, not after the whole prologue.
- Edge rows (|dy| rows outside the image) are zero-filled from a zero
  tile with one batched DMA per dy (all 7 k-blocks at once).
"""

import numpy as np

import concourse.bass as bass
import concourse.mybir as mybir
from concourse.bass_utils import run_bass_kernel_spmd

B, HEADS, C, H, W = 1, 8, 32, 128, 128
WIN = 7
PAD = 3
K = WIN * WIN
FP = H * W  # flat image floats per partition (16384)

_nc_cache = None


def _build_nc():
    f32 = mybir.dt.float32
    nc = bass.Bass()
    x = nc.dram_tensor("x", [C, H, W], f32, kind="ExternalInput")
    y = nc.dram_tensor("y", [K, C, H, W], f32, kind="ExternalOutput")

    with (
        nc.sbuf_tensor("T1", [4 * C, H, W], f32) as T1,
        nc.sbuf_tensor("T2", [4 * C, H, W], f32) as T2,
        nc.sbuf_tensor("Z", [112, 768], f32) as Z,
        nc.semaphore("s_ldS") as s_ldS,
        nc.semaphore("s_ldA") as s_ldA,
        nc.semaphore("s_dve") as s_dve,
        nc.semaphore("s_sp") as s_sp,
        nc.semaphore("s_act") as s_act,
        nc.Block() as block,
    ):
        def band(dx, dy=0):
            """(tensor, first partition) of the dx-shifted flat copy."""
            if dx < 0:
                return T1, 32 * (dx + 3)
            if dx > 0:
                return T2, 32 * dx
            return (T2, 0) if dy % 2 == 0 else (T1, 96)

        # s_dve thresholds at which each band's wrap memset has landed
        BAND_READY = {-1: 2, 1: 3, -2: 4, 2: 5, -3: 6, 3: 7}

        def load_band(eng, dx, sem):
            """Flat (shifted) load of the whole head into band(dx)."""
            buf, p0 = band(dx) if dx != 0 else (
                (T1, 96) if eng is nc.sync else (T2, 0)
            )
            xf = x.rearrange("c h w -> c (h w)")
            if dx >= 0:
                dst = bass.AP(buf, p0 * FP, [[FP, C], [1, FP - dx]])
                src = xf[:, dx:FP]
            else:
                dst = bass.AP(buf, p0 * FP - dx, [[FP, C], [1, FP + dx]])
                src = xf[:, 0 : FP + dx]
            eng.dma_start(out=dst, in_=src).then_inc(sem, 16)

        def edge(eng, dy, sem):
            g = abs(dy)
            k0 = (dy + PAD) * WIN
            r0 = 0 if dy < 0 else H - g
            eng.dma_start(
                out=y[k0 : k0 + WIN, :, r0 : r0 + g, :],
                in_=bass.AP(Z, 0, [[768, 112], [384, 2], [1, 128 * g]]),
            ).then_inc(sem, 16)

        def store(eng, dy, dx, sem):
            n = H - abs(dy)
            ys, yd = max(0, dy), max(0, -dy)
            k = (dy + PAD) * WIN + (dx + PAD)
            buf, p0 = band(dx, dy)
            src = bass.AP(buf, p0 * FP + ys * W, [[FP, C], [1, n * W]])
            dst = y[k, :, yd : yd + n, :]
            eng.dma_start(out=dst, in_=src).then_inc(sem, 16)

        @block.vector
        def _(vector):
            vector.memset(Z[:, :], 0.0).then_inc(s_dve, 1)
            # Gate each band's wrap-column memset on its own load.
            # sync loads: [0dup, -1, -2, -3]; scalar: [0dup, +1, +2, +3]
            for i, dx in enumerate((-1, 1, -2, 2, -3, 3)):
                sem = s_ldS if dx < 0 else s_ldA
                vector.wait_ge(sem, 16 * (abs(dx) + 1))
                buf, p0 = band(dx)
                if dx < 0:
                    ap = buf[p0 : p0 + C, :, 0:-dx]
                else:
                    ap = buf[p0 : p0 + C, :, W - dx : W]
                vector.memset(ap, 0.0).then_inc(s_dve, 1)

        dys = list(range(-PAD, PAD + 1))

        @block.sync
        def _(sync):
            for dx in (0, -1, -2, -3):
                load_band(nc.sync, dx, s_ldS)
            sync.wait_ge(s_dve, 1)
            for dy in (-3, -2, -1):
                edge(nc.sync, dy, s_sp)
            n_st = 0
            # dx=0 stores from the even-parity dup (T2[0:32], scalar's load)
            sync.wait_ge(s_ldA, 16)
            for dy in dys:
                if dy % 2 == 0:
                    store(nc.sync, dy, 0, s_sp)
                    n_st += 1
            # even-parity shifted bands in readiness order
            for dx in (1, -2, -3):
                sync.wait_ge(s_dve, BAND_READY[dx])
                for dy in dys:
                    store(nc.sync, dy, dx, s_sp)
                    n_st += 1
            sync.wait_ge(s_sp, 16 * (3 + n_st))

        @block.scalar
        def _(scalar):
            for dx in (0, 1, 2, 3):
                load_band(nc.scalar, dx, s_ldA)
            scalar.wait_ge(s_dve, 1)
            for dy in (1, 2, 3):
                edge(nc.scalar, dy, s_act)
            n_st = 0
            # dx=0 stores from the odd-parity dup (T1[96:128], sync's load)
            scalar.wait_ge(s_ldS, 16)
            for dy in dys:
                if dy % 2 != 0:
                    store(nc.scalar, dy, 0, s_act)
                    n_st += 1
            for dx in (-1, 2, 3):
                scalar.wait_ge(s_dve, BAND_READY[dx])
                for dy in dys:
                    store(nc.scalar, dy, dx, s_act)
                    n_st += 1
            scalar.wait_ge(s_act, 16 * (3 + n_st))

    return nc


def _get_nc():
    global _nc_cache
    if _nc_cache is None:
        _nc_cache = _build_nc()
    return _nc_cache


def kernel(x: np.ndarray) -> np.ndarray:
    assert x.shape == (B, HEADS, C, H, W), x.shape
    nc = _get_nc()
    in_maps = [
        {"x": np.ascontiguousarray(x[0, h], dtype=np.float32)} for h in range(HEADS)
    ]
    res = run_bass_kernel_spmd(nc, in_maps, core_ids=list(range(HEADS)))
    out = np.stack([res.results[h]["y"] for h in range(HEADS)], axis=0)
    return out[None]  # [1, 8, 49, 32, 128, 128]
